# revision 10
# baseline (speedup 1.0000x reference)
"""Chamfer-distance loss (CCHLoss) kernel for 8 Trainium2 NeuronCores.

Contract: kernel(**inputs) takes the FULL unsharded inputs
  v:        (16, 2048, 3) f32
  v_pred:   (16, 2048, 3) f32
  mask:     (4, 4, 2, 32, 32) f32
  pred_dw:  (16, 2048, 3) f32
and returns (loss, loss_normals) matching reference().

Strategy: data-parallel over the B=16 batch dim, 2 batches per core.
Per batch the 2048x2048 squared-distance matrix is produced by TensorE
via a K=5 matmul (lhsT rows [-2x0,-2x1,-2x2,|x|^2,1], rhs rows
[y0,y1,y2,1,|y|^2]) in float32r.  VectorE reduces it:
  - one fused tensor_tensor_reduce per [128,2048] PSUM group does the
    PSUM->SBUF(bf16) copy AND the row-min (-> cham_pred),
  - a bf16 tensor_tensor min chain accumulates the column-min,
  - PE transposes + reduce fold the 128 partitions (-> cham_v),
  - mask-weighted sums reduce everything to per-core scalars.
Host only shards/permutes inputs and sums 8 cores' partial sums.
"""

import numpy as np

B, P1, P2, D = 16, 2048, 2048, 3
NCORES = 8
BPC = B // NCORES  # batches per core
NT = P1 // 128     # i-tiles per batch
NJ = P2 // 512     # matmul j-chunks per group
NC128 = P2 // 128  # 128-wide j-chunks (transpose fold)

_CACHE = {}


def build_bass():
    """Build + compile the per-core Bass program (same program all 8 cores)."""
    import concourse.bacc as bacc
    import concourse.tile as tile
    from concourse import mybir
    from concourse.masks import make_identity

    f32 = mybir.dt.float32
    bf16 = mybir.dt.bfloat16
    f32r = mybir.dt.float32r
    Alu = mybir.AluOpType
    Act = mybir.ActivationFunctionType
    X = mybir.AxisListType.X

    nc = bacc.Bacc("TRN2", target_bir_lowering=False, debug=False)

    xT_h = nc.dram_tensor("xT", (BPC, 3, P1), f32r, kind="ExternalInput")
    yT_h = nc.dram_tensor("yT", (BPC, 3, P2), f32r, kind="ExternalInput")
    maskT_h = nc.dram_tensor("maskT", (BPC, 128, NC128), f32, kind="ExternalInput")
    dw_h = nc.dram_tensor("dw", (128, BPC * 48), f32, kind="ExternalInput")
    out_h = nc.dram_tensor("out", (1, 8), f32, kind="ExternalOutput")

    with tile.TileContext(nc) as tc:
        with (
            tc.tile_pool(name="consts", bufs=1) as consts,
            tc.tile_pool(name="opnds", bufs=2) as opnds,
            tc.tile_pool(name="scr", bufs=3) as scr,
            tc.tile_pool(name="small", bufs=4) as small,
            tc.tile_pool(name="ps", bufs=2, space="PSUM") as ps,
        ):
            ident = consts.tile([128, 128], bf16)
            make_identity(nc, ident)
            ones128 = consts.tile([128, 1], f32)
            nc.vector.memset(ones128, 1.0)
            # memset can't emit f32r directly — stage in f32, ACT-copy to f32r
            ones_f32 = consts.tile([1, P2], f32)
            nc.vector.memset(ones_f32, 1.0)
            ones_row = consts.tile([1, P2], f32r)
            nc.scalar.copy(ones_row[:], ones_f32[:])
            ones3_f32 = consts.tile([3, 1], f32)
            nc.vector.memset(ones3_f32, 1.0)
            ones3 = consts.tile([3, 1], f32r)
            nc.scalar.copy(ones3[:], ones3_f32[:])
            partials = consts.tile([128, 8], f32)
            nc.vector.memset(partials, 0.0)

            # --- mean(pred_dw^2) partial: ACT square with sum-accumulate ---
            dwt = consts.tile([128, BPC * 48], f32)
            nc.sync.dma_start(out=dwt[:], in_=dw_h[:])
            dwsq = consts.tile([128, BPC * 48], f32)
            nc.scalar.activation(
                out=dwsq[:], in_=dwt[:], func=Act.Square,
                accum_out=partials[:, 6:7],
            )

            for b in range(BPC):
                # ---------- operand prep ----------
                xstage = opnds.tile([3, P1], f32r)
                nc.sync.dma_start(out=xstage[:], in_=xT_h[b])
                lhsT = opnds.tile([5, P1], f32r)
                rhs = opnds.tile([5, P2], f32r)
                nc.sync.dma_start(out=rhs[0:3, :], in_=yT_h[b])
                # ones rows land at partition offsets 3/4 — engines can't
                # address those, so place them with SBUF->SBUF DMA.
                nc.sync.dma_start(out=rhs[3:4, :], in_=ones_row[0:1, :])
                nc.sync.dma_start(out=lhsT[4:5, :], in_=ones_row[0:1, 0:P1])
                # lhsT rows 0-2 = -2 * x
                nc.scalar.activation(
                    out=lhsT[0:3, :], in_=xstage[:], func=Act.Copy, scale=-2.0
                )
                # squares for the norm rows
                xsq = opnds.tile([3, P1], f32r)
                nc.scalar.activation(out=xsq[:], in_=xstage[:], func=Act.Square)
                ysq = opnds.tile([3, P2], f32r)
                nc.scalar.activation(out=ysq[:], in_=rhs[0:3, :], func=Act.Square)
                # |x|^2 row: ones(3,1).T @ xsq via PE, then copy PSUM->lhsT row 3
                ps_n = ps.tile([128, P2], f32, tag="dgrp")
                for c in range(NJ):
                    sl = slice(c * 512, (c + 1) * 512)
                    nc.tensor.matmul(
                        ps_n[0:1, sl], ones3[:],
                        xsq[:, sl],
                    )
                xnrow = opnds.tile([1, P1], f32r)
                nc.scalar.copy(xnrow[:], ps_n[0:1, :])
                nc.sync.dma_start(out=lhsT[3:4, :], in_=xnrow[:])
                ps_n2 = ps.tile([128, P2], f32, tag="dgrp")
                for c in range(NJ):
                    sl = slice(c * 512, (c + 1) * 512)
                    nc.tensor.matmul(
                        ps_n2[0:1, sl], ones3[:],
                        ysq[:, sl],
                    )
                ynrow = opnds.tile([1, P2], f32r)
                nc.scalar.copy(ynrow[:], ps_n2[0:1, :])
                nc.sync.dma_start(out=rhs[4:5, :], in_=ynrow[:])

                # ---------- main distance + min pipeline ----------
                colacc = opnds.tile([128, P2], bf16)
                rowparts = opnds.tile([128, NT, 128], bf16)
                rowaccs = small.tile([128, NT], f32)
                chamv = small.tile([128, NC128], f32)

                for t in range(NT):
                    g = ps.tile([128, P2], f32, tag="dgrp")
                    lsl = lhsT[:, t * 128:(t + 1) * 128]
                    for c in range(NJ):
                        sl = slice(c * 512, (c + 1) * 512)
                        nc.tensor.matmul(g[:, sl], lsl, rhs[:, sl])
                    # ACT evacuates PSUM -> SBUF bf16
                    s = scr.tile([128, P2], bf16)
                    nc.scalar.copy(out=s[:], in_=g[:])
                    # row-min: bf16 2x-mode fold chain 2048 -> 128
                    f1 = scr.tile([128, 1024], bf16, tag="f1")
                    nc.vector.tensor_tensor(
                        out=f1[:], in0=s[:, 0:1024], in1=s[:, 1024:2048],
                        op=Alu.min,
                    )
                    f2 = scr.tile([128, 512], bf16, tag="f2")
                    nc.vector.tensor_tensor(
                        out=f2[:], in0=f1[:, 0:512], in1=f1[:, 512:1024],
                        op=Alu.min,
                    )
                    f3 = scr.tile([128, 256], bf16, tag="f3")
                    nc.vector.tensor_tensor(
                        out=f3[:], in0=f2[:, 0:256], in1=f2[:, 256:512],
                        op=Alu.min,
                    )
                    nc.vector.tensor_tensor(
                        out=rowparts[:, t, :], in0=f3[:, 0:128],
                        in1=f3[:, 128:256], op=Alu.min,
                    )
                    # col-min accumulate
                    if t == 0:
                        nc.vector.tensor_copy(out=colacc[:], in_=s[:])
                    else:
                        nc.vector.tensor_tensor(
                            out=colacc[:], in0=colacc[:], in1=s[:], op=Alu.min
                        )
                # finish row-min: [128, 16, 128] -> [128, 16]
                nc.vector.tensor_reduce(
                    out=rowaccs[:], in_=rowparts[:], axis=X, op=Alu.min
                )

                # ---------- fold colacc partitions via PE transpose ----------
                for r in range(2):
                    tp = ps.tile([128, P2], bf16, tag="dgrp")
                    for cc in range(8):
                        cidx = r * 8 + cc
                        nc.tensor.transpose(
                            tp[:, cc * 128:(cc + 1) * 128],
                            colacc[:, cidx * 128:(cidx + 1) * 128],
                            ident[:],
                        )
                    tpv = tp[:, 0:1024].rearrange("p (a b) -> p a b", b=128)
                    nc.vector.tensor_reduce(
                        out=chamv[:, r * 8:(r + 1) * 8], in_=tpv, axis=X,
                        op=Alu.min,
                    )

                # ---------- per-batch scalars ----------
                mk = small.tile([128, NC128], f32)
                nc.sync.dma_start(out=mk[:], in_=maskT_h[b])
                prod = small.tile([128, NC128], f32)
                nc.vector.tensor_tensor(
                    out=prod[:], in0=chamv[:], in1=mk[:], op=Alu.mult
                )
                nc.vector.tensor_reduce(
                    out=partials[:, 2 * b:2 * b + 1], in_=prod[:], axis=X,
                    op=Alu.add,
                )
                nc.vector.tensor_reduce(
                    out=partials[:, 2 * b + 1:2 * b + 2], in_=rowaccs[:],
                    axis=X, op=Alu.add,
                )

            # ---------- cross-partition sum of all partials via PE ----------
            fin = ps.tile([128, P2], f32, tag="dgrp")
            nc.tensor.matmul(fin[0:1, 0:8], ones128[:], partials[:])
            res = small.tile([1, 8], f32)
            nc.scalar.copy(res[:], fin[0:1, 0:8])
            nc.sync.dma_start(out=out_h[:], in_=res[:])

    nc.compile()
    return nc


def get_compiled():
    if "nc" not in _CACHE:
        _CACHE["nc"] = build_bass()
    return _CACHE["nc"]


def make_in_maps(v, v_pred, mask, pred_dw):
    v = np.asarray(v, np.float32)
    v_pred = np.asarray(v_pred, np.float32)
    mask = np.asarray(mask, np.float32)
    pred_dw = np.asarray(pred_dw, np.float32)

    xT = np.ascontiguousarray(v_pred.transpose(0, 2, 1))  # (16, 3, 2048)
    yT = np.ascontiguousarray(v.transpose(0, 2, 1))
    mask_flat = mask.reshape(B, P2)
    # maskT[b, p, c] = mask_flat[b, c*128 + p]
    maskT = np.ascontiguousarray(
        mask_flat.reshape(B, NC128, 128).transpose(0, 2, 1)
    )
    in_maps = []
    for k in range(NCORES):
        b0 = BPC * k
        dwp = np.concatenate(
            [pred_dw[b0 + i].reshape(128, 48) for i in range(BPC)], axis=1
        )
        in_maps.append({
            "xT": np.ascontiguousarray(xT[b0:b0 + BPC]),
            "yT": np.ascontiguousarray(yT[b0:b0 + BPC]),
            "maskT": np.ascontiguousarray(maskT[b0:b0 + BPC]),
            "dw": np.ascontiguousarray(dwp),
        })
    return in_maps


def combine_outs(outs):
    """outs: (8, 8) array of per-core partial rows -> (loss, loss_normals)."""
    outs = np.asarray(outs, np.float64)
    mcols = [2 * i for i in range(BPC)]
    rcols = [2 * i + 1 for i in range(BPC)]
    msum = outs[:, mcols].sum()
    rsum = outs[:, rcols].sum()
    dsum = outs[:, 6].sum()
    loss = msum / (B * P2) + rsum / (B * P1) + dsum / (B * P1 * D)
    return (np.float32(loss), np.float32(0.0))


def kernel(**inputs):
    from concourse.bass_utils import run_bass_kernel_spmd

    nc = get_compiled()
    in_maps = make_in_maps(
        inputs["v"], inputs["v_pred"], inputs["mask"], inputs["pred_dw"]
    )
    res = run_bass_kernel_spmd(nc, in_maps, core_ids=list(range(NCORES)))
    outs = np.stack([r["out"].reshape(8) for r in res.results])
    return combine_outs(outs)


# revision 12
# speedup vs baseline: 1.0567x; 1.0567x over previous
"""Chamfer-distance loss (CCHLoss) kernel for 8 Trainium2 NeuronCores.

Contract: kernel(**inputs) takes the FULL unsharded inputs
  v:        (16, 2048, 3) f32
  v_pred:   (16, 2048, 3) f32
  mask:     (4, 4, 2, 32, 32) f32
  pred_dw:  (16, 2048, 3) f32
and returns (loss, loss_normals) matching reference().

Strategy: data-parallel over the B=16 batch dim, 2 batches per core.
Per batch the 2048x2048 squared-distance matrix is produced by TensorE
via a K=5 matmul (lhsT rows [-2x0,-2x1,-2x2,|x|^2,1], rhs rows
[y0,y1,y2,1,|y|^2]) in float32r.  VectorE reduces it:
  - one fused tensor_tensor_reduce per [128,2048] PSUM group does the
    PSUM->SBUF(bf16) copy AND the row-min (-> cham_pred),
  - a bf16 tensor_tensor min chain accumulates the column-min,
  - PE transposes + reduce fold the 128 partitions (-> cham_v),
  - mask-weighted sums reduce everything to per-core scalars.
Host only shards/permutes inputs and sums 8 cores' partial sums.
"""

import numpy as np

B, P1, P2, D = 16, 2048, 2048, 3
NCORES = 8
BPC = B // NCORES  # batches per core
NT = P1 // 128     # i-tiles per batch
NJ = P2 // 512     # matmul j-chunks per group
NC128 = P2 // 128  # 128-wide j-chunks (transpose fold)

_CACHE = {}


def build_bass():
    """Build + compile the per-core Bass program (same program all 8 cores)."""
    import concourse.bacc as bacc
    import concourse.tile as tile
    from concourse import mybir
    from concourse.masks import make_identity

    f32 = mybir.dt.float32
    bf16 = mybir.dt.bfloat16
    f32r = mybir.dt.float32r
    Alu = mybir.AluOpType
    Act = mybir.ActivationFunctionType
    X = mybir.AxisListType.X

    nc = bacc.Bacc("TRN2", target_bir_lowering=False, debug=False)

    xprod_h = nc.dram_tensor("xprod", (BPC, 6, P1), bf16, kind="ExternalInput")
    yprod_h = nc.dram_tensor("yprod", (BPC, 6, P2), bf16, kind="ExternalInput")
    cdx_h = nc.dram_tensor("cdx", (BPC, 128, 48), f32, kind="ExternalInput")
    cdy_h = nc.dram_tensor("cdy", (BPC, 128, 48), f32, kind="ExternalInput")
    maskT_h = nc.dram_tensor("maskT", (BPC, 128, NC128), f32, kind="ExternalInput")
    dw_h = nc.dram_tensor("dw", (128, BPC * 48), f32, kind="ExternalInput")
    out_h = nc.dram_tensor("out", (1, 8), f32, kind="ExternalOutput")

    with tile.TileContext(nc) as tc:
        with (
            tc.tile_pool(name="consts", bufs=1) as consts,
            tc.tile_pool(name="opnds", bufs=2) as opnds,
            tc.tile_pool(name="scr", bufs=3) as scr,
            tc.tile_pool(name="small", bufs=4) as small,
            tc.tile_pool(name="ps", bufs=2, space="PSUM") as ps,
        ):
            ident = consts.tile([128, 128], bf16)
            make_identity(nc, ident)
            ones128 = consts.tile([128, 1], f32)
            nc.vector.memset(ones128, 1.0)
            ones_row = consts.tile([1, P2], bf16)
            nc.vector.memset(ones_row, 1.0)
            partials = consts.tile([128, 8], f32)
            nc.vector.memset(partials, 0.0)

            # --- mean(pred_dw^2) partial: ACT square with sum-accumulate ---
            dwt = consts.tile([128, BPC * 48], f32)
            nc.sync.dma_start(out=dwt[:], in_=dw_h[:])
            dwsq = consts.tile([128, BPC * 48], f32)
            nc.scalar.activation(
                out=dwsq[:], in_=dwt[:], func=Act.Square,
                accum_out=partials[:, 6:7],
            )

            for b in range(BPC):
                # ---------- operand prep (K=13 bf16 hi/lo split) ----------
                # norm rows, computed in [128,48] layout (cheap ops), then
                # DMA'd into the operand rows in identity j-order.
                lhsT = opnds.tile([13, P1], bf16)
                rhs = opnds.tile([13, P2], bf16)
                nrm_rows = []
                for side, cd_h in (("x", cdx_h), ("y", cdy_h)):
                    cd = opnds.tile([128, 48], f32, tag=f"cd{side}")
                    nc.sync.dma_start(out=cd[:], in_=cd_h[b])
                    sq = opnds.tile([128, 48], f32, tag=f"sq{side}")
                    nc.scalar.activation(out=sq[:], in_=cd[:], func=Act.Square)
                    nrm = opnds.tile([128, 16], f32, tag=f"nrm{side}")
                    nc.vector.tensor_reduce(
                        out=nrm[:], in_=sq[:].rearrange("p (n d) -> p n d", d=3),
                        axis=X, op=Alu.add,
                    )
                    nh = opnds.tile([128, 16], bf16, tag=f"nh{side}")
                    nc.scalar.copy(nh[:], nrm[:])
                    nl = opnds.tile([128, 16], bf16, tag=f"nl{side}")
                    nc.vector.tensor_tensor(
                        out=nl[:], in0=nrm[:], in1=nh[:], op=Alu.subtract
                    )
                    nrm_rows.append((nh, nl))
                (xnh, xnl), (ynh, ynl) = nrm_rows
                # lhsT rows: [wh x3, wh x3, wl x3, x2nh, x2nl, 1, 1]  (w = -2x)
                nc.sync.dma_start(out=lhsT[0:3, :], in_=xprod_h[b, 0:3])
                nc.sync.dma_start(out=lhsT[3:6, :], in_=xprod_h[b, 0:3])
                nc.sync.dma_start(out=lhsT[6:9, :], in_=xprod_h[b, 3:6])
                nc.sync.dma_start(out=lhsT[9:10, :], in_=xnh[:])
                nc.sync.dma_start(out=lhsT[10:11, :], in_=xnl[:])
                nc.sync.dma_start(out=lhsT[11:12, :], in_=ones_row[0:1, :])
                nc.sync.dma_start(out=lhsT[12:13, :], in_=ones_row[0:1, :])
                # rhs rows: [yh x3, yl x3, yh x3, 1, 1, y2nh, y2nl]
                nc.sync.dma_start(out=rhs[0:3, :], in_=yprod_h[b, 0:3])
                nc.sync.dma_start(out=rhs[3:6, :], in_=yprod_h[b, 3:6])
                nc.sync.dma_start(out=rhs[6:9, :], in_=yprod_h[b, 0:3])
                nc.sync.dma_start(out=rhs[9:10, :], in_=ones_row[0:1, :])
                nc.sync.dma_start(out=rhs[10:11, :], in_=ones_row[0:1, :])
                nc.sync.dma_start(out=rhs[11:12, :], in_=ynh[:])
                nc.sync.dma_start(out=rhs[12:13, :], in_=ynl[:])

                # ---------- main distance + min pipeline ----------
                colacc = opnds.tile([128, P2], bf16)
                rowparts = opnds.tile([128, NT, 128], bf16)
                rowaccs = small.tile([128, NT], f32)
                chamv = small.tile([128, NC128], f32)

                for t in range(NT):
                    g = ps.tile([128, P2], f32, tag="dgrp")
                    lsl = lhsT[:, t * 128:(t + 1) * 128]
                    for c in range(NJ):
                        sl = slice(c * 512, (c + 1) * 512)
                        nc.tensor.matmul(g[:, sl], lsl, rhs[:, sl])
                    # ACT evacuates PSUM -> SBUF bf16
                    s = scr.tile([128, P2], bf16)
                    nc.scalar.copy(out=s[:], in_=g[:])
                    # row-min: bf16 2x-mode fold chain 2048 -> 128
                    f1 = scr.tile([128, 1024], bf16, tag="f1")
                    nc.vector.tensor_tensor(
                        out=f1[:], in0=s[:, 0:1024], in1=s[:, 1024:2048],
                        op=Alu.min,
                    )
                    f2 = scr.tile([128, 512], bf16, tag="f2")
                    nc.vector.tensor_tensor(
                        out=f2[:], in0=f1[:, 0:512], in1=f1[:, 512:1024],
                        op=Alu.min,
                    )
                    f3 = scr.tile([128, 256], bf16, tag="f3")
                    nc.vector.tensor_tensor(
                        out=f3[:], in0=f2[:, 0:256], in1=f2[:, 256:512],
                        op=Alu.min,
                    )
                    nc.vector.tensor_tensor(
                        out=rowparts[:, t, :], in0=f3[:, 0:128],
                        in1=f3[:, 128:256], op=Alu.min,
                    )
                    # col-min accumulate
                    if t == 0:
                        nc.vector.tensor_copy(out=colacc[:], in_=s[:])
                    else:
                        nc.vector.tensor_tensor(
                            out=colacc[:], in0=colacc[:], in1=s[:], op=Alu.min
                        )
                # finish row-min: [128, 16, 128] -> [128, 16]
                nc.vector.tensor_reduce(
                    out=rowaccs[:], in_=rowparts[:], axis=X, op=Alu.min
                )

                # ---------- fold colacc partitions via PE transpose ----------
                for r in range(2):
                    tp = ps.tile([128, P2], bf16, tag="dgrp")
                    for cc in range(8):
                        cidx = r * 8 + cc
                        nc.tensor.transpose(
                            tp[:, cc * 128:(cc + 1) * 128],
                            colacc[:, cidx * 128:(cidx + 1) * 128],
                            ident[:],
                        )
                    tpv = tp[:, 0:1024].rearrange("p (a b) -> p a b", b=128)
                    nc.vector.tensor_reduce(
                        out=chamv[:, r * 8:(r + 1) * 8], in_=tpv, axis=X,
                        op=Alu.min,
                    )

                # ---------- per-batch scalars ----------
                mk = small.tile([128, NC128], f32)
                nc.sync.dma_start(out=mk[:], in_=maskT_h[b])
                prod = small.tile([128, NC128], f32)
                nc.vector.tensor_tensor(
                    out=prod[:], in0=chamv[:], in1=mk[:], op=Alu.mult
                )
                nc.vector.tensor_reduce(
                    out=partials[:, 2 * b:2 * b + 1], in_=prod[:], axis=X,
                    op=Alu.add,
                )
                nc.vector.tensor_reduce(
                    out=partials[:, 2 * b + 1:2 * b + 2], in_=rowaccs[:],
                    axis=X, op=Alu.add,
                )

            # ---------- cross-partition sum of all partials via PE ----------
            fin = ps.tile([128, P2], f32, tag="dgrp")
            nc.tensor.matmul(fin[0:1, 0:8], ones128[:], partials[:])
            res = small.tile([1, 8], f32)
            nc.scalar.copy(res[:], fin[0:1, 0:8])
            nc.sync.dma_start(out=out_h[:], in_=res[:])

    nc.compile()
    return nc


def get_compiled():
    if "nc" not in _CACHE:
        _CACHE["nc"] = build_bass()
    return _CACHE["nc"]


def make_in_maps(v, v_pred, mask, pred_dw):
    import ml_dtypes

    bf16 = ml_dtypes.bfloat16
    v = np.asarray(v, np.float32)
    v_pred = np.asarray(v_pred, np.float32)
    mask = np.asarray(mask, np.float32)
    pred_dw = np.asarray(pred_dw, np.float32)

    # lossless bf16 hi/lo repacking of the matmul operands
    wT = (-2.0 * v_pred).transpose(0, 2, 1)           # (16, 3, 2048) f32
    wh = wT.astype(bf16)
    wl = (wT - wh.astype(np.float32)).astype(bf16)
    xprod = np.concatenate([wh, wl], axis=1)          # (16, 6, 2048) bf16
    yT = v.transpose(0, 2, 1)
    yh = yT.astype(bf16)
    yl = (yT - yh.astype(np.float32)).astype(bf16)
    yprod = np.concatenate([yh, yl], axis=1)

    cdx = v_pred.reshape(B, 128, 48)
    cdy = v.reshape(B, 128, 48)
    mask_flat = mask.reshape(B, P2)
    # maskT[b, p, c] = mask_flat[b, c*128 + p]
    maskT = np.ascontiguousarray(
        mask_flat.reshape(B, NC128, 128).transpose(0, 2, 1)
    )
    in_maps = []
    for k in range(NCORES):
        b0 = BPC * k
        dwp = np.concatenate(
            [pred_dw[b0 + i].reshape(128, 48) for i in range(BPC)], axis=1
        )
        in_maps.append({
            "xprod": np.ascontiguousarray(xprod[b0:b0 + BPC]),
            "yprod": np.ascontiguousarray(yprod[b0:b0 + BPC]),
            "cdx": np.ascontiguousarray(cdx[b0:b0 + BPC]),
            "cdy": np.ascontiguousarray(cdy[b0:b0 + BPC]),
            "maskT": np.ascontiguousarray(maskT[b0:b0 + BPC]),
            "dw": np.ascontiguousarray(dwp),
        })
    return in_maps


def combine_outs(outs):
    """outs: (8, 8) array of per-core partial rows -> (loss, loss_normals)."""
    outs = np.asarray(outs, np.float64)
    mcols = [2 * i for i in range(BPC)]
    rcols = [2 * i + 1 for i in range(BPC)]
    msum = outs[:, mcols].sum()
    rsum = outs[:, rcols].sum()
    dsum = outs[:, 6].sum()
    loss = msum / (B * P2) + rsum / (B * P1) + dsum / (B * P1 * D)
    return (np.float32(loss), np.float32(0.0))


def kernel(**inputs):
    from concourse.bass_utils import run_bass_kernel_spmd

    nc = get_compiled()
    in_maps = make_in_maps(
        inputs["v"], inputs["v_pred"], inputs["mask"], inputs["pred_dw"]
    )
    res = run_bass_kernel_spmd(nc, in_maps, core_ids=list(range(NCORES)))
    outs = np.stack([r["out"].reshape(8) for r in res.results])
    return combine_outs(outs)


# revision 13
# speedup vs baseline: 1.1763x; 1.1132x over previous
"""Chamfer-distance loss (CCHLoss) kernel for 8 Trainium2 NeuronCores.

Contract: kernel(**inputs) takes the FULL unsharded inputs
  v:        (16, 2048, 3) f32
  v_pred:   (16, 2048, 3) f32
  mask:     (4, 4, 2, 32, 32) f32
  pred_dw:  (16, 2048, 3) f32
and returns (loss, loss_normals) matching reference().

Strategy: data-parallel over the B=16 batch dim, 2 batches per core.
Per batch the 2048x2048 squared-distance matrix is produced by TensorE
via a K=5 matmul (lhsT rows [-2x0,-2x1,-2x2,|x|^2,1], rhs rows
[y0,y1,y2,1,|y|^2]) in float32r.  VectorE reduces it:
  - one fused tensor_tensor_reduce per [128,2048] PSUM group does the
    PSUM->SBUF(bf16) copy AND the row-min (-> cham_pred),
  - a bf16 tensor_tensor min chain accumulates the column-min,
  - PE transposes + reduce fold the 128 partitions (-> cham_v),
  - mask-weighted sums reduce everything to per-core scalars.
Host only shards/permutes inputs and sums 8 cores' partial sums.
"""

import numpy as np

B, P1, P2, D = 16, 2048, 2048, 3
NCORES = 8
BPC = B // NCORES  # batches per core
NT = P1 // 128     # i-tiles per batch
NJ = P2 // 512     # matmul j-chunks per group
NC128 = P2 // 128  # 128-wide j-chunks (transpose fold)

_CACHE = {}


def build_bass():
    """Build + compile the per-core Bass program (same program all 8 cores)."""
    import concourse.bacc as bacc
    import concourse.tile as tile
    from concourse import mybir
    from concourse.masks import make_identity

    f32 = mybir.dt.float32
    bf16 = mybir.dt.bfloat16
    f32r = mybir.dt.float32r
    Alu = mybir.AluOpType
    Act = mybir.ActivationFunctionType
    X = mybir.AxisListType.X

    nc = bacc.Bacc("TRN2", target_bir_lowering=False, debug=False)

    xprod_h = nc.dram_tensor("xprod", (BPC, 9, P1), bf16, kind="ExternalInput")
    yprod_h = nc.dram_tensor("yprod", (BPC, 9, P2), bf16, kind="ExternalInput")
    cdx_h = nc.dram_tensor("cdx", (BPC, 128, 48), f32, kind="ExternalInput")
    cdy_h = nc.dram_tensor("cdy", (BPC, 128, 48), f32, kind="ExternalInput")
    maskT_h = nc.dram_tensor("maskT", (BPC, 128, NC128), f32, kind="ExternalInput")
    dw_h = nc.dram_tensor("dw", (128, BPC * 48), f32, kind="ExternalInput")
    out_h = nc.dram_tensor("out", (1, 8), f32, kind="ExternalOutput")

    with tile.TileContext(nc) as tc:
        with (
            tc.tile_pool(name="consts", bufs=1) as consts,
            tc.tile_pool(name="opnds", bufs=2) as opnds,
            tc.tile_pool(name="scr", bufs=3) as scr,
            tc.tile_pool(name="small", bufs=4) as small,
            tc.tile_pool(name="ps", bufs=2, space="PSUM") as ps,
        ):
            ident = consts.tile([128, 128], bf16)
            make_identity(nc, ident)
            ones128 = consts.tile([128, 1], f32)
            nc.vector.memset(ones128, 1.0)
            ones_row = consts.tile([2, P2], bf16)
            nc.vector.memset(ones_row, 1.0)
            partials = consts.tile([128, 8], f32)
            nc.vector.memset(partials, 0.0)

            # --- mean(pred_dw^2) partial: ACT square with sum-accumulate ---
            dwt = consts.tile([128, BPC * 48], f32)
            nc.sync.dma_start(out=dwt[:], in_=dw_h[:])
            dwsq = consts.tile([128, BPC * 48], f32)
            nc.scalar.activation(
                out=dwsq[:], in_=dwt[:], func=Act.Square,
                accum_out=partials[:, 6:7],
            )

            colaccs, rowparts_l = [], []
            for b in range(BPC):
                # ---------- operand prep (K=13 bf16 hi/lo split) ----------
                # norm rows, computed in [128,48] layout (cheap ops), then
                # DMA'd into the operand rows in identity j-order.
                lhsT = opnds.tile([13, P1], bf16)
                rhs = opnds.tile([13, P2], bf16)
                nrm_rows = []
                for side, cd_h in (("x", cdx_h), ("y", cdy_h)):
                    cd = opnds.tile([128, 48], f32, tag=f"cd{side}")
                    nc.sync.dma_start(out=cd[:], in_=cd_h[b])
                    sq = opnds.tile([128, 48], f32, tag=f"sq{side}")
                    nc.scalar.activation(out=sq[:], in_=cd[:], func=Act.Square)
                    nrm = opnds.tile([128, 16], f32, tag=f"nrm{side}")
                    nc.vector.tensor_reduce(
                        out=nrm[:], in_=sq[:].rearrange("p (n d) -> p n d", d=3),
                        axis=X, op=Alu.add,
                    )
                    nh = opnds.tile([128, 16], bf16, tag=f"nh{side}")
                    nc.scalar.copy(nh[:], nrm[:])
                    nl = opnds.tile([128, 16], bf16, tag=f"nl{side}")
                    nc.vector.tensor_tensor(
                        out=nl[:], in0=nrm[:], in1=nh[:], op=Alu.subtract
                    )
                    nrm_rows.append((nh, nl))
                (xnh, xnl), (ynh, ynl) = nrm_rows
                # lhsT rows: [wh x3, wl x3, wh x3, x2nh, x2nl, 1, 1]  (w = -2x)
                # rhs rows:  [yh x3, yh x3, yl x3, 1, 1, y2nh, y2nl]
                # (host pre-duplicates the 9 product rows -> one DMA each)
                nc.sync.dma_start(out=lhsT[0:9, :], in_=xprod_h[b])
                nc.sync.dma_start(out=lhsT[9:10, :], in_=xnh[:])
                nc.sync.dma_start(out=lhsT[10:11, :], in_=xnl[:])
                nc.sync.dma_start(out=lhsT[11:13, :], in_=ones_row[:])
                nc.sync.dma_start(out=rhs[0:9, :], in_=yprod_h[b])
                nc.sync.dma_start(out=rhs[9:11, :], in_=ones_row[:])
                nc.sync.dma_start(out=rhs[11:12, :], in_=ynh[:])
                nc.sync.dma_start(out=rhs[12:13, :], in_=ynl[:])

                # ---------- main distance + min pipeline ----------
                colacc = opnds.tile([128, P2], bf16)
                rowparts = opnds.tile([128, NT, 128], bf16)
                colaccs.append(colacc)
                rowparts_l.append(rowparts)

                for tp_ in range(NT // 2):
                    # two i-tile groups per pipeline step
                    s2 = scr.tile([128, 2, P2], bf16)
                    for u in range(2):
                        t = 2 * tp_ + u
                        g = ps.tile([128, P2], f32, tag="dgrp")
                        lsl = lhsT[:, t * 128:(t + 1) * 128]
                        for c in range(NJ):
                            sl = slice(c * 512, (c + 1) * 512)
                            nc.tensor.matmul(g[:, sl], lsl, rhs[:, sl])
                        # ACT evacuates PSUM -> SBUF bf16
                        nc.scalar.copy(out=s2[:, u, :], in_=g[:])
                    # merged row-min fold chain over both groups
                    f1 = scr.tile([128, 2, 1024], bf16, tag="f1")
                    nc.vector.tensor_tensor(
                        out=f1[:], in0=s2[:, :, 0:1024], in1=s2[:, :, 1024:2048],
                        op=Alu.min,
                    )
                    f2 = scr.tile([128, 2, 512], bf16, tag="f2")
                    nc.vector.tensor_tensor(
                        out=f2[:], in0=f1[:, :, 0:512], in1=f1[:, :, 512:1024],
                        op=Alu.min,
                    )
                    f3 = scr.tile([128, 2, 256], bf16, tag="f3")
                    nc.vector.tensor_tensor(
                        out=f3[:], in0=f2[:, :, 0:256], in1=f2[:, :, 256:512],
                        op=Alu.min,
                    )
                    nc.vector.tensor_tensor(
                        out=rowparts[:, 2 * tp_:2 * tp_ + 2, :],
                        in0=f3[:, :, 0:128], in1=f3[:, :, 128:256], op=Alu.min,
                    )
                    # col-min accumulate: pair-min (independent) then chain
                    m = scr.tile([128, P2], bf16, tag="m")
                    nc.vector.tensor_tensor(
                        out=m[:], in0=s2[:, 0, :], in1=s2[:, 1, :], op=Alu.min
                    )
                    if tp_ == 0:
                        nc.vector.tensor_copy(out=colacc[:], in_=m[:])
                    else:
                        nc.vector.tensor_tensor(
                            out=colacc[:], in0=colacc[:], in1=m[:], op=Alu.min
                        )

            # ---------- deferred per-batch reductions ----------
            for b in range(BPC):
                colacc = colaccs[b]
                rowparts = rowparts_l[b]
                rowaccs = small.tile([128, NT], f32)
                chamv = small.tile([128, NC128], f32)
                # finish row-min: [128, 16, 128] -> [128, 16]
                nc.vector.tensor_reduce(
                    out=rowaccs[:], in_=rowparts[:], axis=X, op=Alu.min
                )
                # fold colacc partitions via PE transpose
                for r in range(2):
                    tp = ps.tile([128, P2], bf16, tag="dgrp")
                    for cc in range(8):
                        cidx = r * 8 + cc
                        nc.tensor.transpose(
                            tp[:, cc * 128:(cc + 1) * 128],
                            colacc[:, cidx * 128:(cidx + 1) * 128],
                            ident[:],
                        )
                    tpv = tp[:, 0:1024].rearrange("p (a b) -> p a b", b=128)
                    nc.vector.tensor_reduce(
                        out=chamv[:, r * 8:(r + 1) * 8], in_=tpv, axis=X,
                        op=Alu.min,
                    )
                # per-batch scalars
                mk = small.tile([128, NC128], f32)
                nc.sync.dma_start(out=mk[:], in_=maskT_h[b])
                prod = small.tile([128, NC128], f32)
                nc.vector.tensor_tensor(
                    out=prod[:], in0=chamv[:], in1=mk[:], op=Alu.mult
                )
                nc.vector.tensor_reduce(
                    out=partials[:, 2 * b:2 * b + 1], in_=prod[:], axis=X,
                    op=Alu.add,
                )
                nc.vector.tensor_reduce(
                    out=partials[:, 2 * b + 1:2 * b + 2], in_=rowaccs[:],
                    axis=X, op=Alu.add,
                )

            # ---------- cross-partition sum of all partials via PE ----------
            fin = ps.tile([128, P2], f32, tag="dgrp")
            nc.tensor.matmul(fin[0:1, 0:8], ones128[:], partials[:])
            res = small.tile([1, 8], f32)
            nc.scalar.copy(res[:], fin[0:1, 0:8])
            nc.sync.dma_start(out=out_h[:], in_=res[:])

    nc.compile()
    return nc


def get_compiled():
    if "nc" not in _CACHE:
        _CACHE["nc"] = build_bass()
    return _CACHE["nc"]


def make_in_maps(v, v_pred, mask, pred_dw):
    import ml_dtypes

    bf16 = ml_dtypes.bfloat16
    v = np.asarray(v, np.float32)
    v_pred = np.asarray(v_pred, np.float32)
    mask = np.asarray(mask, np.float32)
    pred_dw = np.asarray(pred_dw, np.float32)

    # lossless bf16 hi/lo repacking of the matmul operands
    wT = (-2.0 * v_pred).transpose(0, 2, 1)           # (16, 3, 2048) f32
    wh = wT.astype(bf16)
    wl = (wT - wh.astype(np.float32)).astype(bf16)
    xprod = np.concatenate([wh, wl, wh], axis=1)      # (16, 9, 2048) bf16
    yT = v.transpose(0, 2, 1)
    yh = yT.astype(bf16)
    yl = (yT - yh.astype(np.float32)).astype(bf16)
    yprod = np.concatenate([yh, yh, yl], axis=1)

    cdx = v_pred.reshape(B, 128, 48)
    cdy = v.reshape(B, 128, 48)
    mask_flat = mask.reshape(B, P2)
    # maskT[b, p, c] = mask_flat[b, c*128 + p]
    maskT = np.ascontiguousarray(
        mask_flat.reshape(B, NC128, 128).transpose(0, 2, 1)
    )
    in_maps = []
    for k in range(NCORES):
        b0 = BPC * k
        dwp = np.concatenate(
            [pred_dw[b0 + i].reshape(128, 48) for i in range(BPC)], axis=1
        )
        in_maps.append({
            "xprod": np.ascontiguousarray(xprod[b0:b0 + BPC]),
            "yprod": np.ascontiguousarray(yprod[b0:b0 + BPC]),
            "cdx": np.ascontiguousarray(cdx[b0:b0 + BPC]),
            "cdy": np.ascontiguousarray(cdy[b0:b0 + BPC]),
            "maskT": np.ascontiguousarray(maskT[b0:b0 + BPC]),
            "dw": np.ascontiguousarray(dwp),
        })
    return in_maps


def combine_outs(outs):
    """outs: (8, 8) array of per-core partial rows -> (loss, loss_normals)."""
    outs = np.asarray(outs, np.float64)
    mcols = [2 * i for i in range(BPC)]
    rcols = [2 * i + 1 for i in range(BPC)]
    msum = outs[:, mcols].sum()
    rsum = outs[:, rcols].sum()
    dsum = outs[:, 6].sum()
    loss = msum / (B * P2) + rsum / (B * P1) + dsum / (B * P1 * D)
    return (np.float32(loss), np.float32(0.0))


def kernel(**inputs):
    from concourse.bass_utils import run_bass_kernel_spmd

    nc = get_compiled()
    in_maps = make_in_maps(
        inputs["v"], inputs["v_pred"], inputs["mask"], inputs["pred_dw"]
    )
    res = run_bass_kernel_spmd(nc, in_maps, core_ids=list(range(NCORES)))
    outs = np.stack([r["out"].reshape(8) for r in res.results])
    return combine_outs(outs)
